# revision 17
# baseline (speedup 1.0000x reference)
"""Multi-head attention (B=2, F=T=2048, H=1024, 16 heads x 64) on 8 TRN2
NeuronCores.

Head/tensor parallelism: core c owns heads {2c, 2c+1} for both batches and
runs projections + attention for them over the full sequences. Output is
redistributed with four small AllToAlls (one per batch-half) to an
f-chunk-sharded layout (core r owns 128-row f-chunks r and r+8 of each
batch), where the output projection contracts the full 1024 head dims.

Single fused emission stream per core:
  - inputs land via chunked, host-prearranged contiguous DMAs so the first
    projection matmul starts ~4us in,
  - attention for (b=0, fc=0) starts as soon as K/V chunk 0 + Q fc0 are
    projected; remaining projection work (Q b0, K/V/Q b1) is injected as
    "rounds" into the PE slack of the ACT(exp)-paced attention loop,
  - softmax: exp folds the 1/8 logit scale; denominators ride a ones-column
    in V; each 512-wide f-chunk's PSUM accumulator is evacuated to SBUF by
    one DVE copy (freeing the bank), then normalized off-path with
    reciprocal_approx_fast -> ones-matmul broadcast -> DVE multiply,
  - each AllToAll fires as soon as its half-batch of shards is flushed; the
    matching output-projection rounds are injected into later attention
    slack, so only the final collective + 2 rounds are exposed.
All matmuls bf16 with fp32 PSUM accumulation.
"""

from contextlib import ExitStack

import ml_dtypes
import numpy as np

import concourse.bass as bass  # noqa: F401
import concourse.mybir as mybir
import concourse.tile as tile
from concourse import bacc
from concourse.bass_utils import run_bass_kernel_spmd

B, F, T, HID, NH, DH = 2, 2048, 2048, 1024, 16, 64
HT = HID // 128  # 8 h-tiles
TT = T // 128  # 16 key tiles
FC = F // 512  # 4 query chunks per batch
CT = T // 512  # 4 key/value projection chunks
BF16, F32 = mybir.dt.bfloat16, mybir.dt.float32
NPBF16 = ml_dtypes.bfloat16

_CACHE: dict = {}


def _build():
    nc = bacc.Bacc("TRN2", target_bir_lowering=False, debug=False, num_devices=8)

    # host-prearranged inputs: [p, chunk, a, n] with n contiguous per chunk
    s0 = nc.declare_dram_parameter("s0", [128, CT, HT, 512], BF16, isOutput=False)
    s1 = nc.declare_dram_parameter("s1", [128, CT, HT, 512], BF16, isOutput=False)
    q0 = nc.declare_dram_parameter("q0", [128, FC, HT, 512], BF16, isOutput=False)
    q1 = nc.declare_dram_parameter("q1", [128, FC, HT, 512], BF16, isOutput=False)
    w3 = nc.declare_dram_parameter("w3", [128, HT, 3, 128], BF16, isOutput=False)
    wo = nc.declare_dram_parameter("wo", [128, HT, HID], BF16, isOutput=False)
    ident = nc.declare_dram_parameter("ident", [128, 128], BF16, isOutput=False)
    out = nc.declare_dram_parameter("out", [B, 2, 128, HID], F32, isOutput=True)

    seg = 128 * 128  # one A^T shard row: [128 hd, 128 f]
    a2a_in = [
        [nc.dram_tensor(f"a2a_in{b}{h}", [8, seg], BF16) for h in range(2)]
        for b in range(B)
    ]
    a2a_out = [
        [nc.dram_tensor(f"a2a_out{b}{h}", [8, seg], BF16) for h in range(2)]
        for b in range(B)
    ]

    with tile.TileContext(nc) as tc, ExitStack() as ctx:
        persist = ctx.enter_context(tc.tile_pool(name="persist", bufs=1))
        w3_sb = persist.tile([128, HT, 3, 128], BF16, tag="w3")
        wo_sb = persist.tile([128, HT, HID], BF16, tag="wo")
        ones_sb = persist.tile([65, 64], BF16, tag="ones")
        ident_sb = persist.tile([128, 128], BF16, tag="ident")
        kT = [
            [persist.tile([128, 512], BF16, tag=f"kT{b}{ct}", name=f"kT{b}{ct}") for ct in range(CT)]
            for b in range(B)
        ]
        vv = [
            [persist.tile([128, 4, 2, DH + 1], BF16, tag=f"v{b}{ct}", name=f"v{b}{ct}") for ct in range(CT)]
            for b in range(B)
        ]
        qp = [
            [persist.tile([128, 512], BF16, tag=f"qp{b}{fc}", name=f"qp{b}{fc}") for fc in range(FC)]
            for b in range(B)
        ]
        atg = [
            [persist.tile([128, HT, 128], BF16, tag=f"atg{b}{h}", name=f"atg{b}{h}") for h in range(2)]
            for b in range(B)
        ]

        nc.vector.memset(ones_sb[:, :], 1.0)
        for b in range(B):
            for ct in range(CT):
                nc.vector.memset(vv[b][ct][:, :, :, DH : DH + 1], 1.0)

        for a in range(0, HT, 2):
            nc.sync.dma_start(
                out=w3_sb[:, a : a + 2, :, :], in_=w3[:, a : a + 2, :, :]
            )
        nc.sync.dma_start(out=ident_sb[:, :], in_=ident[:, :])

        with (
            tc.tile_pool(name="stg", bufs=12) as stg_pool,
            tc.tile_pool(name="ptp", bufs=6) as pt_pool,
            tc.tile_pool(name="abp", bufs=2) as ab_pool,
            tc.tile_pool(name="rbp", bufs=2) as rb_pool,
            tc.tile_pool(name="vts", bufs=2) as vts_pool,
            tc.tile_pool(name="bcs", bufs=2) as bcs_pool,
            tc.tile_pool(name="stp", bufs=4) as st_pool,
            tc.tile_pool(name="otp", bufs=2) as ot_pool,
            tc.tile_pool(name="s_ps", bufs=2, space="PSUM") as s_ps_pool,
            tc.tile_pool(name="a_ps", bufs=2, space="PSUM") as a_ps_pool,
            tc.tile_pool(name="bc_ps", bufs=1, space="PSUM") as bc_ps_pool,
            tc.tile_pool(name="pj_ps", bufs=1, space="PSUM") as pj_ps_pool,
        ):
            # ---- staged input DMAs (order matches consumption order) ----
            chunks = {}
            dma_order = [
                ("s", 0, 0), ("q", 0, 0), ("s", 0, 1), ("s", 0, 2), ("s", 0, 3),
                ("q", 0, 1), ("q", 0, 2), ("q", 0, 3),
                ("s", 1, 0), ("s", 1, 1), ("s", 1, 2), ("s", 1, 3),
                ("q", 1, 0), ("q", 1, 1), ("q", 1, 2), ("q", 1, 3),
            ]
            srcs = {("s", 0): s0, ("s", 1): s1, ("q", 0): q0, ("q", 1): q1}
            for idx, (kind, b, c) in enumerate(dma_order):
                t = stg_pool.tile([128, HT, 512], BF16, tag="stg", name=f"c_{kind}{b}{c}")
                w = 2 if idx < 4 else 4
                for a in range(0, HT, w):
                    nc.sync.dma_start(
                        out=t[:, a : a + w, :],
                        in_=srcs[(kind, b)][:, c, a : a + w, :],
                    )
                chunks[(kind, b, c)] = t
            nc.sync.dma_start(out=wo_sb[:, :, :], in_=wo[:, :, :])

            # ---- projection / output-projection rounds -----------------
            def k_round(b, ct):
                ps = pj_ps_pool.tile([128, 512], F32, tag="pj", name="kps")
                st = chunks[("s", b, ct)]
                for ht in range(HT):
                    nc.tensor.matmul(
                        ps[:, :],
                        lhsT=w3_sb[:, ht, 1, :],
                        rhs=st[:, ht, :],
                        start=(ht == 0),
                        stop=(ht == HT - 1),
                    )
                nc.vector.tensor_copy(out=kT[b][ct][:, :], in_=ps[:, :])

            vts = {}

            def va_round(b, ct):
                ps = pj_ps_pool.tile([128, 512], F32, tag="pj", name="vps")
                st = chunks[("s", b, ct)]
                for ht in range(HT):
                    nc.tensor.matmul(
                        ps[:, :],
                        lhsT=w3_sb[:, ht, 2, :],
                        rhs=st[:, ht, :],
                        start=(ht == 0),
                        stop=(ht == HT - 1),
                    )
                vt = vts_pool.tile([128, 512], BF16, tag="vt", name="vt")
                nc.vector.tensor_copy(out=vt[:, :], in_=ps[:, :])
                vts[(b, ct)] = vt

            def vb_round(b, ct):
                vt = vts.pop((b, ct))
                for i in range(4):
                    tp = pj_ps_pool.tile([128, 128], BF16, tag="pj", name="tp")
                    nc.tensor.transpose(
                        tp[:, :], vt[:, 128 * i : 128 * (i + 1)], ident_sb[:, :]
                    )
                    nc.vector.tensor_copy(
                        out=vv[b][ct][:, i, :, 0:DH],
                        in_=tp[:, :].rearrange("p (j d) -> p j d", j=2),
                    )

            def q_round(b, fc):
                ps = pj_ps_pool.tile([128, 512], F32, tag="pj", name="qps")
                qt = chunks[("q", b, fc)]
                for ht in range(HT):
                    nc.tensor.matmul(
                        ps[:, :],
                        lhsT=w3_sb[:, ht, 0, :],
                        rhs=qt[:, ht, :],
                        start=(ht == 0),
                        stop=(ht == HT - 1),
                    )
                nc.vector.tensor_copy(out=qp[b][fc][:, :], in_=ps[:, :])

            def op_round(b, h, j, pool=None):
                ps = (pool or pj_ps_pool).tile(
                    [128, 512], F32, tag="bc" if pool else "pj", name="ops"
                )
                at = atg[b][h]
                for a in range(HT):
                    nc.tensor.matmul(
                        ps[:, :],
                        lhsT=at[:, a, :],
                        rhs=wo_sb[:, a, 512 * j : 512 * (j + 1)],
                        start=(a == 0),
                        stop=(a == HT - 1),
                    )
                ot = ot_pool.tile([128, 512], F32, tag="ot", name="ot")
                nc.vector.tensor_copy(out=ot[:, :], in_=ps[:, :])
                for k in range(2):
                    nc.sync.dma_start(
                        out=out[b, h, :, 512 * j + 256 * k : 512 * j + 256 * (k + 1)],
                        in_=ot[:, 256 * k : 256 * (k + 1)],
                    )

            # ---- softmax chunk evacuation + normalization --------------
            def ab_evac(a_cur):
                ab = ab_pool.tile([65, 2, 512], F32, tag="ab", name="ab")
                rb = rb_pool.tile([65, 2, 512], BF16, tag="rb", name="rb")
                for j in range(2):
                    nc.vector.tensor_copy(
                        out=rb[64:65, j, :], in_=a_cur[j][64:65, :]
                    )
                for j in range(2):
                    nc.vector.tensor_copy(out=ab[:, j, :], in_=a_cur[j][:, :])
                return (ab, rb)

            def flush(b, fc, abrb):
                ab, rb = abrb
                half = fc // 2
                st2 = st_pool.tile([128, 512], BF16, tag="st", name="st2")
                for j in range(2):
                    # bf16 ones-matmul broadcasts the denominator row to 64
                    # partitions; approx-reciprocal runs at partition base 0
                    # (it silently corrupts at base 64).
                    bc = bc_ps_pool.tile([64, 512], F32, tag="bc", name="bc")
                    nc.tensor.matmul(
                        bc[:, :],
                        lhsT=ones_sb[64:65, :],
                        rhs=rb[64:65, j, :],
                        start=True,
                        stop=True,
                    )
                    bcc = bcs_pool.tile([64, 512], F32, tag="bcc", name="bcc")
                    nc.vector.reciprocal_approx_fast(out=bcc[:, :], in_=bc[:, :])
                    nc.vector.tensor_mul(
                        out=st2[64 * j : 64 * (j + 1), :],
                        in0=ab[0:64, j, :],
                        in1=bcc[:, :],
                    )
                r0 = (4 * fc) % 8
                rows = a2a_in[b][half][r0 : r0 + 4, :].rearrange(
                    "r (p n) -> p r n", p=128
                )
                nc.sync.dma_start(
                    out=rows,
                    in_=st2[:, :].rearrange("p (r n) -> p r n", r=4),
                )

            def collective(b, h):
                nc.gpsimd.collective_compute(
                    "AllToAll",
                    mybir.AluOpType.bypass,
                    replica_groups=[[0, 1, 2, 3, 4, 5, 6, 7]],
                    ins=[a2a_in[b][h].ap().opt()],
                    outs=[a2a_out[b][h].ap().opt()],
                )

            def atg_load(b, h):
                # deferred: emitted once the collective is (nearly) done so
                # this DMA never head-of-line-blocks the sync queue
                src_v = a2a_out[b][h][:, :].rearrange("a (p n) -> p a n", p=128)
                for a in range(0, HT, 4):
                    nc.sync.dma_start(
                        out=atg[b][h][:, a : a + 4, :], in_=src_v[:, a : a + 4, :]
                    )

            # ---- attention pipeline ------------------------------------
            def emit_s_exp(b, fc, tt):
                kc = kT[b][tt // 4]
                qc = qp[b][fc]
                i = tt % 4
                sp = s_ps_pool.tile([128, 2, 512], F32, tag="s", name="sp")
                for j in range(2):
                    nc.tensor.matmul(
                        sp[:, j, :],
                        lhsT=kc[64 * j : 64 * (j + 1), 128 * i : 128 * (i + 1)],
                        rhs=qc[64 * j : 64 * (j + 1), :],
                        start=True,
                        stop=True,
                    )
                pt = pt_pool.tile([128, 2, 512], BF16, tag="pt", name="pt")
                nc.scalar.activation(
                    out=pt[:, :, :],
                    in_=sp[:, :, :],
                    func=mybir.ActivationFunctionType.Exp,
                    scale=float(DH) ** -0.5,
                )
                return pt

            # events[i]: emitted after step i's PV matmuls
            events = {i: [] for i in range(128)}

            def at(i, fn, *args):
                events[i].append((fn, args))

            # b0 K/V chunks 1-3 land just ahead of the steps that need them
            at(0, k_round, 0, 1)
            at(1, va_round, 0, 1)
            at(2, vb_round, 0, 1)
            at(3, k_round, 0, 2)
            at(4, va_round, 0, 2)
            at(5, vb_round, 0, 2)
            at(6, k_round, 0, 3)
            at(7, va_round, 0, 3)
            at(8, vb_round, 0, 3)
            at(9, q_round, 0, 1)
            at(19, q_round, 0, 2)
            at(35, q_round, 0, 3)
            # b1 projections during b0 attention
            at(36, k_round, 1, 0)
            at(39, va_round, 1, 0)
            at(41, vb_round, 1, 0)
            at(42, k_round, 1, 1)
            at(45, va_round, 1, 1)
            at(47, vb_round, 1, 1)
            at(48, k_round, 1, 2)
            at(51, va_round, 1, 2)
            at(53, vb_round, 1, 2)
            at(54, k_round, 1, 3)
            at(57, va_round, 1, 3)
            at(59, vb_round, 1, 3)
            at(60, q_round, 1, 0)
            at(68, q_round, 1, 1)
            at(84, q_round, 1, 2)
            at(100, q_round, 1, 3)
            # output-projection rounds as AllToAll halves land
            at(62, op_round, 0, 0, 0)
            at(64, op_round, 0, 0, 1)
            at(78, op_round, 0, 1, 0)
            at(82, op_round, 0, 1, 1)
            at(110, op_round, 1, 0, 0)
            at(120, op_round, 1, 0, 1)
            at(55, atg_load, 0, 0)
            at(74, atg_load, 0, 1)
            at(106, atg_load, 1, 0)

            # warm the PE p-state while the first input chunks stream in
            for _ in range(40):
                wps = pj_ps_pool.tile([2, 2], F32, tag="pj", name="wps")
                nc.tensor.matmul(
                    wps[:, :],
                    lhsT=ones_sb[0:1, 0:2],
                    rhs=ones_sb[0:1, 0:2],
                    start=True,
                    stop=True,
                )

            # prologue: K + Q for chunk 0, first S/EXP in flight before the
            # V rounds so the ACT engine starts as early as possible
            steps = [
                (b, fc, tt) for b in range(B) for fc in range(FC)
                for tt in range(TT)
            ]
            k_round(0, 0)
            q_round(0, 0)
            pend_flush = None
            pts = {}
            a_cur = None
            pts[steps[0]] = emit_s_exp(*steps[0])
            va_round(0, 0)
            vb_round(0, 0)
            for i, (b, fc, tt) in enumerate(steps):
                if tt == 0:
                    a_cur = [
                        a_ps_pool.tile([65, 512], F32, tag="a", name=f"a_acc{j}")
                        for j in range(2)
                    ]
                if i + 1 < len(steps):
                    pts[steps[i + 1]] = emit_s_exp(*steps[i + 1])
                pt = pts.pop((b, fc, tt))
                for j in range(2):
                    nc.tensor.matmul(
                        a_cur[j][:, :],
                        lhsT=vv[b][tt // 4][:, tt % 4, j, :],
                        rhs=pt[:, j, :],
                        start=(tt == 0),
                        stop=(tt == TT - 1),
                    )
                if tt == 2 and pend_flush is not None:
                    flush(*pend_flush)
                    pend_flush = None
                if tt == TT - 1:
                    ab = ab_evac(a_cur)
                    if fc == FC - 1:
                        # end of batch: flush + final collective inline
                        flush(b, fc, ab)
                        collective(b, 1)
                    else:
                        pend_flush = (b, fc, ab)
                for fn, args in events[i]:
                    fn(*args)
                # first-half collective fires once fc0+fc1 are flushed
                if fc == 2 and tt == 3:
                    collective(b, 0)

            # final output-projection rounds (need the last collective);
            # keep the PE p-state up through the collective wait
            for _ in range(60):
                wps = pj_ps_pool.tile([2, 2], F32, tag="pj", name="wps")
                nc.tensor.matmul(
                    wps[:, :],
                    lhsT=ones_sb[0:1, 0:2],
                    rhs=ones_sb[0:1, 0:2],
                    start=True,
                    stop=True,
                )
            atg_load(1, 1)
            op_round(1, 1, 0)
            op_round(1, 1, 1, pool=bc_ps_pool)

    nc.compile()
    return nc


def _get_nc():
    if "nc" not in _CACHE:
        _CACHE["nc"] = _build()
    return _CACHE["nc"]


def _reference_fallback(query_input, source_input, bias, wq, wk, wv, wo):
    """Numpy fallback, only used if bias is unexpectedly nonzero."""
    q = np.einsum("bfh,hnd->bfnd", query_input, wq) * (DH**-0.5)
    k = np.einsum("bth,hnd->btnd", source_input, wk)
    v = np.einsum("bth,hnd->btnd", source_input, wv)
    logits = np.einsum("btnd,bfnd->bnft", k, q) + bias
    logits -= logits.max(axis=-1, keepdims=True)
    w = np.exp(logits)
    w /= w.sum(axis=-1, keepdims=True)
    attn = np.einsum("bnft,btnd->bfnd", w, v)
    return np.einsum("bfnd,ndh->bfh", attn, wo).astype(np.float32)


def _chunked(x2d):
    # [L, HID] -> [128p, chunk, a, 512n]
    return np.ascontiguousarray(
        x2d.reshape(4, 512, HT, 128).transpose(3, 0, 2, 1)
    ).astype(NPBF16)


def make_in_maps(query_input, source_input, wq, wk, wv, wo):
    wo2 = wo.reshape(HID, HID)
    wo_h = np.ascontiguousarray(
        wo2.reshape(HT, 128, HID).transpose(1, 0, 2)
    ).astype(NPBF16)
    ident_h = np.ascontiguousarray(np.eye(128, dtype=np.float32)).astype(NPBF16)
    s_h = [_chunked(source_input[b]) for b in range(B)]
    q_h = [_chunked(query_input[b]) for b in range(B)]
    wqh = wq.reshape(HID, NH, DH)
    wkh = wk.reshape(HID, NH, DH)
    wvh = wv.reshape(HID, NH, DH)

    in_maps = []
    for c in range(8):
        sl = np.s_[:, 2 * c : 2 * c + 2, :]
        w3c = np.stack(
            [
                wqh[sl].reshape(HID, 128),
                wkh[sl].reshape(HID, 128),
                wvh[sl].reshape(HID, 128),
            ],
            axis=1,
        )  # [HID, 3, 128]
        w3c = np.ascontiguousarray(
            w3c.reshape(HT, 128, 3, 128).transpose(1, 0, 2, 3)
        ).astype(NPBF16)
        in_maps.append(
            {
                "s0": s_h[0],
                "s1": s_h[1],
                "q0": q_h[0],
                "q1": q_h[1],
                "w3": w3c,
                "wo": wo_h,
                "ident": ident_h,
            }
        )
    return in_maps


def assemble(results):
    out_full = np.empty((B, F, HID), dtype=np.float32)
    for r in range(8):
        o = results[r]["out"]  # [B, 2, 128, HID]
        for b in range(B):
            out_full[b, 128 * r : 128 * (r + 1), :] = o[b, 0]
            out_full[b, 1024 + 128 * r : 1024 + 128 * (r + 1), :] = o[b, 1]
    return out_full


def kernel(query_input, source_input, bias, wq, wk, wv, wo):
    query_input = np.asarray(query_input, dtype=np.float32)
    source_input = np.asarray(source_input, dtype=np.float32)
    bias = np.asarray(bias, dtype=np.float32)
    wq = np.asarray(wq, dtype=np.float32)
    wk = np.asarray(wk, dtype=np.float32)
    wv = np.asarray(wv, dtype=np.float32)
    wo = np.asarray(wo, dtype=np.float32)

    if np.any(bias):
        return _reference_fallback(query_input, source_input, bias, wq, wk, wv, wo)

    in_maps = make_in_maps(query_input, source_input, wq, wk, wv, wo)
    nc = _get_nc()
    res = run_bass_kernel_spmd(nc, in_maps, core_ids=list(range(8)))
    return assemble(res.results)


# revision 19
# speedup vs baseline: 1.0934x; 1.0934x over previous
"""Multi-head attention (B=2, F=T=2048, H=1024, 16 heads x 64) on 8 TRN2
NeuronCores.

Head/tensor parallelism: core c owns heads {2c, 2c+1} for both batches and
runs projections + attention for them over the full sequences. Output is
redistributed with four small AllToAlls (one per batch-half) to an
f-chunk-sharded layout (core r owns 128-row f-chunks r and r+8 of each
batch), where the output projection contracts the full 1024 head dims.

Single fused emission stream per core:
  - inputs land via chunked, host-prearranged contiguous DMAs so the first
    projection matmul starts ~4us in,
  - attention for (b=0, fc=0) starts as soon as K/V chunk 0 + Q fc0 are
    projected; remaining projection work (Q b0, K/V/Q b1) is injected as
    "rounds" into the PE slack of the ACT(exp)-paced attention loop,
  - softmax: exp folds the 1/8 logit scale; denominators ride a ones-column
    in V; each 512-wide f-chunk's PSUM accumulator is evacuated to SBUF by
    one DVE copy (freeing the bank), then normalized off-path with
    reciprocal_approx_fast -> ones-matmul broadcast -> DVE multiply,
  - each AllToAll fires as soon as its half-batch of shards is flushed; the
    matching output-projection rounds are injected into later attention
    slack, so only the final collective + 2 rounds are exposed.
All matmuls bf16 with fp32 PSUM accumulation.
"""

from contextlib import ExitStack

import ml_dtypes
import numpy as np

import concourse.bass as bass  # noqa: F401
import concourse.mybir as mybir
import concourse.tile as tile
from concourse import bacc
from concourse.bass_utils import run_bass_kernel_spmd

B, F, T, HID, NH, DH = 2, 2048, 2048, 1024, 16, 64
HT = HID // 128  # 8 h-tiles
TT = T // 128  # 16 key tiles
FC = F // 512  # 4 query chunks per batch
CT = T // 512  # 4 key/value projection chunks
BF16, F32 = mybir.dt.bfloat16, mybir.dt.float32
NPBF16 = ml_dtypes.bfloat16

_CACHE: dict = {}


def _build():
    nc = bacc.Bacc("TRN2", target_bir_lowering=False, debug=False, num_devices=8)

    # host-prearranged inputs: [p, chunk, a, n] with n contiguous per chunk
    s0 = nc.declare_dram_parameter("s0", [128, CT, HT, 512], BF16, isOutput=False)
    s1 = nc.declare_dram_parameter("s1", [128, CT, HT, 512], BF16, isOutput=False)
    q0 = nc.declare_dram_parameter("q0", [128, FC, HT, 512], BF16, isOutput=False)
    q1 = nc.declare_dram_parameter("q1", [128, FC, HT, 512], BF16, isOutput=False)
    w3 = nc.declare_dram_parameter("w3", [128, HT, 3, 128], BF16, isOutput=False)
    wo = nc.declare_dram_parameter("wo", [128, HT, HID], BF16, isOutput=False)
    ident = nc.declare_dram_parameter("ident", [128, 128], BF16, isOutput=False)
    out = nc.declare_dram_parameter("out", [B, 2, 128, HID], F32, isOutput=True)

    seg = 128 * 128  # one A^T shard row: [128 hd, 128 f]
    a2a_in = [
        [nc.dram_tensor(f"a2a_in{b}{h}", [8, seg], BF16) for h in range(2)]
        for b in range(B)
    ]
    a2a_out = [
        [nc.dram_tensor(f"a2a_out{b}{h}", [8, seg], BF16) for h in range(2)]
        for b in range(B)
    ]

    with tile.TileContext(nc) as tc, ExitStack() as ctx:
        persist = ctx.enter_context(tc.tile_pool(name="persist", bufs=1))
        w3_sb = persist.tile([128, HT, 3, 128], BF16, tag="w3")
        wo_sb = persist.tile([128, HT, HID], BF16, tag="wo")
        ones_sb = persist.tile([65, 64], BF16, tag="ones")
        ident_sb = persist.tile([128, 128], BF16, tag="ident")
        kT = [
            [persist.tile([128, 512], BF16, tag=f"kT{b}{ct}", name=f"kT{b}{ct}") for ct in range(CT)]
            for b in range(B)
        ]
        vv = [
            [persist.tile([128, 4, 2, DH + 1], BF16, tag=f"v{b}{ct}", name=f"v{b}{ct}") for ct in range(CT)]
            for b in range(B)
        ]
        qp = [
            [persist.tile([128, 512], BF16, tag=f"qp{b}{fc}", name=f"qp{b}{fc}") for fc in range(FC)]
            for b in range(B)
        ]
        atg = [
            [persist.tile([128, HT, 128], BF16, tag=f"atg{b}{h}", name=f"atg{b}{h}") for h in range(2)]
            for b in range(B)
        ]

        nc.vector.memset(ones_sb[:, :], 1.0)
        for b in range(B):
            for ct in range(CT):
                nc.vector.memset(vv[b][ct][:, :, :, DH : DH + 1], 1.0)

        for a in range(0, HT, 2):
            nc.sync.dma_start(
                out=w3_sb[:, a : a + 2, :, :], in_=w3[:, a : a + 2, :, :]
            )
        nc.sync.dma_start(out=ident_sb[:, :], in_=ident[:, :])

        with (
            tc.tile_pool(name="stg", bufs=12) as stg_pool,
            tc.tile_pool(name="ptp", bufs=6) as pt_pool,
            tc.tile_pool(name="abp", bufs=2) as ab_pool,
            tc.tile_pool(name="rbp", bufs=2) as rb_pool,
            tc.tile_pool(name="vts", bufs=2) as vts_pool,
            tc.tile_pool(name="bcs", bufs=2) as bcs_pool,
            tc.tile_pool(name="stp", bufs=4) as st_pool,
            tc.tile_pool(name="otp", bufs=2) as ot_pool,
            tc.tile_pool(name="s_ps", bufs=2, space="PSUM") as s_ps_pool,
            tc.tile_pool(name="a_ps", bufs=2, space="PSUM") as a_ps_pool,
            tc.tile_pool(name="bc_ps", bufs=1, space="PSUM") as bc_ps_pool,
            tc.tile_pool(name="pj_ps", bufs=1, space="PSUM") as pj_ps_pool,
        ):
            # ---- staged input DMAs (order matches consumption order) ----
            chunks = {}
            dma_order = [
                ("s", 0, 0), ("q", 0, 0), ("s", 0, 1), ("s", 0, 2), ("s", 0, 3),
                ("q", 0, 1), ("q", 0, 2), ("q", 0, 3),
                ("s", 1, 0), ("s", 1, 1), ("s", 1, 2), ("s", 1, 3),
                ("q", 1, 0), ("q", 1, 1), ("q", 1, 2), ("q", 1, 3),
            ]
            srcs = {("s", 0): s0, ("s", 1): s1, ("q", 0): q0, ("q", 1): q1}
            for idx, (kind, b, c) in enumerate(dma_order):
                t = stg_pool.tile([128, HT, 512], BF16, tag="stg", name=f"c_{kind}{b}{c}")
                w = 2 if idx < 4 else 4
                for a in range(0, HT, w):
                    nc.sync.dma_start(
                        out=t[:, a : a + w, :],
                        in_=srcs[(kind, b)][:, c, a : a + w, :],
                    )
                chunks[(kind, b, c)] = t
            nc.sync.dma_start(out=wo_sb[:, :, :], in_=wo[:, :, :])

            # ---- projection / output-projection rounds -----------------
            def k_round(b, ct):
                ps = pj_ps_pool.tile([128, 512], F32, tag="pj", name="kps")
                st = chunks[("s", b, ct)]
                for ht in range(HT):
                    nc.tensor.matmul(
                        ps[:, :],
                        lhsT=w3_sb[:, ht, 1, :],
                        rhs=st[:, ht, :],
                        start=(ht == 0),
                        stop=(ht == HT - 1),
                    )
                nc.vector.tensor_copy(out=kT[b][ct][:, :], in_=ps[:, :])

            vts = {}

            def va_round(b, ct):
                ps = pj_ps_pool.tile([128, 512], F32, tag="pj", name="vps")
                st = chunks[("s", b, ct)]
                for ht in range(HT):
                    nc.tensor.matmul(
                        ps[:, :],
                        lhsT=w3_sb[:, ht, 2, :],
                        rhs=st[:, ht, :],
                        start=(ht == 0),
                        stop=(ht == HT - 1),
                    )
                vt = vts_pool.tile([128, 512], BF16, tag="vt", name="vt")
                nc.vector.tensor_copy(out=vt[:, :], in_=ps[:, :])
                vts[(b, ct)] = vt

            def vb_round(b, ct):
                vt = vts.pop((b, ct))
                for i in range(4):
                    tp = pj_ps_pool.tile([128, 128], BF16, tag="pj", name="tp")
                    nc.tensor.transpose(
                        tp[:, :], vt[:, 128 * i : 128 * (i + 1)], ident_sb[:, :]
                    )
                    nc.vector.tensor_copy(
                        out=vv[b][ct][:, i, :, 0:DH],
                        in_=tp[:, :].rearrange("p (j d) -> p j d", j=2),
                    )

            def q_round(b, fc, pool=None):
                ps = (pool or pj_ps_pool).tile(
                    [128, 512], F32, tag="bc" if pool else "pj", name="qps"
                )
                qt = chunks[("q", b, fc)]
                for ht in range(HT):
                    nc.tensor.matmul(
                        ps[:, :],
                        lhsT=w3_sb[:, ht, 0, :],
                        rhs=qt[:, ht, :],
                        start=(ht == 0),
                        stop=(ht == HT - 1),
                    )
                nc.vector.tensor_copy(out=qp[b][fc][:, :], in_=ps[:, :])

            def op_round(b, h, j, pool=None):
                ps = (pool or pj_ps_pool).tile(
                    [128, 512], F32, tag="bc" if pool else "pj", name="ops"
                )
                at = atg[b][h]
                for a in range(HT):
                    nc.tensor.matmul(
                        ps[:, :],
                        lhsT=at[:, a, :],
                        rhs=wo_sb[:, a, 512 * j : 512 * (j + 1)],
                        start=(a == 0),
                        stop=(a == HT - 1),
                    )
                ot = ot_pool.tile([128, 512], F32, tag="ot", name="ot")
                nc.vector.tensor_copy(out=ot[:, :], in_=ps[:, :])
                for k in range(2):
                    nc.sync.dma_start(
                        out=out[b, h, :, 512 * j + 256 * k : 512 * j + 256 * (k + 1)],
                        in_=ot[:, 256 * k : 256 * (k + 1)],
                    )

            # ---- softmax chunk evacuation + normalization --------------
            def ab_evac(a_cur):
                ab = ab_pool.tile([65, 2, 512], F32, tag="ab", name="ab")
                rb = rb_pool.tile([65, 2, 512], BF16, tag="rb", name="rb")
                for j in range(2):
                    nc.vector.tensor_copy(
                        out=rb[64:65, j, :], in_=a_cur[j][64:65, :]
                    )
                for j in range(2):
                    nc.vector.tensor_copy(out=ab[:, j, :], in_=a_cur[j][:, :])
                return (ab, rb)

            def flush(b, fc, abrb):
                ab, rb = abrb
                half = fc // 2
                st2 = st_pool.tile([128, 512], BF16, tag="st", name="st2")
                for j in range(2):
                    # bf16 ones-matmul broadcasts the denominator row to 64
                    # partitions; approx-reciprocal runs at partition base 0
                    # (it silently corrupts at base 64).
                    bc = bc_ps_pool.tile([64, 512], F32, tag="bc", name="bc")
                    nc.tensor.matmul(
                        bc[:, :],
                        lhsT=ones_sb[64:65, :],
                        rhs=rb[64:65, j, :],
                        start=True,
                        stop=True,
                    )
                    bcc = bcs_pool.tile([64, 512], F32, tag="bcc", name="bcc")
                    nc.vector.reciprocal_approx_fast(out=bcc[:, :], in_=bc[:, :])
                    nc.vector.tensor_mul(
                        out=st2[64 * j : 64 * (j + 1), :],
                        in0=ab[0:64, j, :],
                        in1=bcc[:, :],
                    )
                r0 = (4 * fc) % 8
                rows = a2a_in[b][half][r0 : r0 + 4, :].rearrange(
                    "r (p n) -> p r n", p=128
                )
                st2v = st2[:, :].rearrange("p (r n) -> p r n", r=4)
                for j in range(2):
                    nc.sync.dma_start(
                        out=rows[64 * j : 64 * (j + 1), :, :],
                        in_=st2v[64 * j : 64 * (j + 1), :, :],
                    )

            def collective(b, h):
                nc.gpsimd.collective_compute(
                    "AllToAll",
                    mybir.AluOpType.bypass,
                    replica_groups=[[0, 1, 2, 3, 4, 5, 6, 7]],
                    ins=[a2a_in[b][h].ap().opt()],
                    outs=[a2a_out[b][h].ap().opt()],
                )

            def atg_load(b, h):
                # deferred: emitted once the collective is (nearly) done so
                # this DMA never head-of-line-blocks the sync queue
                src_v = a2a_out[b][h][:, :].rearrange("a (p n) -> p a n", p=128)
                for a in range(0, HT, 4):
                    nc.sync.dma_start(
                        out=atg[b][h][:, a : a + 4, :], in_=src_v[:, a : a + 4, :]
                    )

            # ---- attention pipeline ------------------------------------
            def emit_s_exp(b, fc, tt):
                kc = kT[b][tt // 4]
                qc = qp[b][fc]
                i = tt % 4
                sp = s_ps_pool.tile([128, 2, 512], F32, tag="s", name="sp")
                for j in range(2):
                    nc.tensor.matmul(
                        sp[:, j, :],
                        lhsT=kc[64 * j : 64 * (j + 1), 128 * i : 128 * (i + 1)],
                        rhs=qc[64 * j : 64 * (j + 1), :],
                        start=True,
                        stop=True,
                    )
                pt = pt_pool.tile([128, 2, 512], BF16, tag="pt", name="pt")
                nc.scalar.activation(
                    out=pt[:, :, :],
                    in_=sp[:, :, :],
                    func=mybir.ActivationFunctionType.Exp,
                    scale=float(DH) ** -0.5,
                )
                return pt

            # events[i]: emitted after step i's PV matmuls
            events = {i: [] for i in range(128)}

            def at(i, fn, *args):
                events[i].append((fn, args))

            # b0 K/V chunks 1-3 land just ahead of the steps that need them
            at(0, k_round, 0, 1)
            at(1, va_round, 0, 1)
            at(2, vb_round, 0, 1)
            at(3, k_round, 0, 2)
            at(4, va_round, 0, 2)
            at(5, vb_round, 0, 2)
            at(6, k_round, 0, 3)
            at(7, va_round, 0, 3)
            at(8, vb_round, 0, 3)
            at(9, q_round, 0, 1)
            at(19, q_round, 0, 2)
            at(35, q_round, 0, 3)
            # b1 projections during b0 attention
            at(36, k_round, 1, 0)
            at(39, va_round, 1, 0)
            at(41, vb_round, 1, 0)
            at(42, k_round, 1, 1)
            at(45, va_round, 1, 1)
            at(47, vb_round, 1, 1)
            at(48, k_round, 1, 2)
            at(51, va_round, 1, 2)
            at(53, vb_round, 1, 2)
            at(54, k_round, 1, 3)
            at(57, va_round, 1, 3)
            at(59, vb_round, 1, 3)
            at(60, q_round, 1, 0)
            at(68, q_round, 1, 1)
            at(84, q_round, 1, 2)
            at(100, q_round, 1, 3)
            # output-projection rounds as AllToAll halves land
            at(62, op_round, 0, 0, 0)
            at(64, op_round, 0, 0, 1)
            at(78, op_round, 0, 1, 0)
            at(82, op_round, 0, 1, 1)
            at(110, op_round, 1, 0, 0)
            at(120, op_round, 1, 0, 1)
            at(55, atg_load, 0, 0)
            at(74, atg_load, 0, 1)
            at(106, atg_load, 1, 0)

            # warm the PE p-state while the first input chunks stream in
            for _ in range(40):
                wps = pj_ps_pool.tile([2, 2], F32, tag="pj", name="wps")
                nc.tensor.matmul(
                    wps[:, :],
                    lhsT=ones_sb[0:1, 0:2],
                    rhs=ones_sb[0:1, 0:2],
                    start=True,
                    stop=True,
                )

            # prologue: K + Q for chunk 0, first S/EXP in flight before the
            # V rounds so the ACT engine starts as early as possible
            steps = [
                (b, fc, tt) for b in range(B) for fc in range(FC)
                for tt in range(TT)
            ]
            k_round(0, 0)
            q_round(0, 0, pool=bc_ps_pool)
            pend_flush = None
            pts = {}
            a_cur = None
            pts[steps[0]] = emit_s_exp(*steps[0])
            va_round(0, 0)
            vb_round(0, 0)
            for i, (b, fc, tt) in enumerate(steps):
                if tt == 0:
                    a_cur = [
                        a_ps_pool.tile([65, 512], F32, tag="a", name=f"a_acc{j}")
                        for j in range(2)
                    ]
                if i + 1 < len(steps):
                    pts[steps[i + 1]] = emit_s_exp(*steps[i + 1])
                pt = pts.pop((b, fc, tt))
                for j in range(2):
                    nc.tensor.matmul(
                        a_cur[j][:, :],
                        lhsT=vv[b][tt // 4][:, tt % 4, j, :],
                        rhs=pt[:, j, :],
                        start=(tt == 0),
                        stop=(tt == TT - 1),
                    )
                if tt == 2 and pend_flush is not None:
                    flush(*pend_flush)
                    pend_flush = None
                if tt == TT - 1:
                    ab = ab_evac(a_cur)
                    if fc == FC - 1:
                        # end of batch: flush + final collective inline
                        flush(b, fc, ab)
                        collective(b, 1)
                    else:
                        pend_flush = (b, fc, ab)
                for fn, args in events[i]:
                    fn(*args)
                # first-half collective fires once fc0+fc1 are flushed
                if fc == 2 and tt == 3:
                    collective(b, 0)

            # final output-projection rounds (need the last collective);
            # keep the PE p-state up through the collective wait
            for _ in range(60):
                wps = pj_ps_pool.tile([2, 2], F32, tag="pj", name="wps")
                nc.tensor.matmul(
                    wps[:, :],
                    lhsT=ones_sb[0:1, 0:2],
                    rhs=ones_sb[0:1, 0:2],
                    start=True,
                    stop=True,
                )
            atg_load(1, 1)
            op_round(1, 1, 0)
            op_round(1, 1, 1, pool=bc_ps_pool)

    nc.compile()
    return nc


def _get_nc():
    if "nc" not in _CACHE:
        _CACHE["nc"] = _build()
    return _CACHE["nc"]


def _reference_fallback(query_input, source_input, bias, wq, wk, wv, wo):
    """Numpy fallback, only used if bias is unexpectedly nonzero."""
    q = np.einsum("bfh,hnd->bfnd", query_input, wq) * (DH**-0.5)
    k = np.einsum("bth,hnd->btnd", source_input, wk)
    v = np.einsum("bth,hnd->btnd", source_input, wv)
    logits = np.einsum("btnd,bfnd->bnft", k, q) + bias
    logits -= logits.max(axis=-1, keepdims=True)
    w = np.exp(logits)
    w /= w.sum(axis=-1, keepdims=True)
    attn = np.einsum("bnft,btnd->bfnd", w, v)
    return np.einsum("bfnd,ndh->bfh", attn, wo).astype(np.float32)


def _chunked(x2d):
    # [L, HID] -> [128p, chunk, a, 512n]
    return np.ascontiguousarray(
        x2d.reshape(4, 512, HT, 128).transpose(3, 0, 2, 1)
    ).astype(NPBF16)


def make_in_maps(query_input, source_input, wq, wk, wv, wo):
    wo2 = wo.reshape(HID, HID)
    wo_h = np.ascontiguousarray(
        wo2.reshape(HT, 128, HID).transpose(1, 0, 2)
    ).astype(NPBF16)
    ident_h = np.ascontiguousarray(np.eye(128, dtype=np.float32)).astype(NPBF16)
    s_h = [_chunked(source_input[b]) for b in range(B)]
    q_h = [_chunked(query_input[b]) for b in range(B)]
    wqh = wq.reshape(HID, NH, DH)
    wkh = wk.reshape(HID, NH, DH)
    wvh = wv.reshape(HID, NH, DH)

    in_maps = []
    for c in range(8):
        sl = np.s_[:, 2 * c : 2 * c + 2, :]
        w3c = np.stack(
            [
                wqh[sl].reshape(HID, 128),
                wkh[sl].reshape(HID, 128),
                wvh[sl].reshape(HID, 128),
            ],
            axis=1,
        )  # [HID, 3, 128]
        w3c = np.ascontiguousarray(
            w3c.reshape(HT, 128, 3, 128).transpose(1, 0, 2, 3)
        ).astype(NPBF16)
        in_maps.append(
            {
                "s0": s_h[0],
                "s1": s_h[1],
                "q0": q_h[0],
                "q1": q_h[1],
                "w3": w3c,
                "wo": wo_h,
                "ident": ident_h,
            }
        )
    return in_maps


def assemble(results):
    out_full = np.empty((B, F, HID), dtype=np.float32)
    for r in range(8):
        o = results[r]["out"]  # [B, 2, 128, HID]
        for b in range(B):
            out_full[b, 128 * r : 128 * (r + 1), :] = o[b, 0]
            out_full[b, 1024 + 128 * r : 1024 + 128 * (r + 1), :] = o[b, 1]
    return out_full


def kernel(query_input, source_input, bias, wq, wk, wv, wo):
    query_input = np.asarray(query_input, dtype=np.float32)
    source_input = np.asarray(source_input, dtype=np.float32)
    bias = np.asarray(bias, dtype=np.float32)
    wq = np.asarray(wq, dtype=np.float32)
    wk = np.asarray(wk, dtype=np.float32)
    wv = np.asarray(wv, dtype=np.float32)
    wo = np.asarray(wo, dtype=np.float32)

    if np.any(bias):
        return _reference_fallback(query_input, source_input, bias, wq, wk, wv, wo)

    in_maps = make_in_maps(query_input, source_input, wq, wk, wv, wo)
    nc = _get_nc()
    res = run_bass_kernel_spmd(nc, in_maps, core_ids=list(range(8)))
    return assemble(res.results)
